# revision 1
# baseline (speedup 1.0000x reference)
"""Trainium2 Bass kernel for nn_ChemicalDevelopment (drag-scan + separable
Gaussian blur + mask-combine + 3x3 channel coupling + tanh saturation).

Self-contained: hardcodes shapes/sharding. Shards the W (column) axis across
8 NeuronCores with a 12-column halo; each core processes its full-height
column slab independently (no collectives).

Per-core algorithm, natural layout [H-rows on partitions, (w,c) on free]:
  - causal row scan  soft = (1-d)*L X   -> PE: lower-tri Toeplitz T per
    128-row block + 64-row history matrix U against the previous block
  - vertical blur    hardv = Kv X      -> PE: band matrix B0 + 32-row halo
    tiles Bup/Bdn against neighbour blocks (exact 25-tap kernel)
  - horizontal blur  hard = Kh hardv   -> DVE: shifted adds (radius RH)
  - inhibitor u = soft + (X*s)*(hard-soft)            -> DVE
  - v_j = X_j - sum_i C[i,j] u_i  (9 strided STT ops) -> DVE
  - out = 3*tanh(s*v)                                 -> ACT
"""
import numpy as np

H_FULL = 4096
W_FULL = 4096
NCORES = 8
WS = W_FULL // NCORES      # 512 columns per core
HALO = 12                  # blur halo (25-tap kernel -> radius 12)
P = 128                    # partition block (rows)
RH = 3                     # horizontal blur taps kept each side
HIST = 64                  # scan history rows from previous block
SIGMA_SOFT = 2.0
SIGMA_HARD = 0.5
D_MAX = 3.0
SINV = 1.0 / (D_MAX + 1e-6)
MMCHUNK = 512              # max fp32 matmul moving free dim / PSUM bank

_NC_CACHE = {}


def _taps64():
    # identical arithmetic to the reference (f32), then f64 for matrix build
    x = np.arange(-12, 13, dtype=np.float32)
    k = np.exp(np.float32(-0.5) * (x / np.float32(SIGMA_HARD)) ** 2)
    k = k / k.sum()
    return k.astype(np.float64)


def _matrices():
    d = np.exp(-1.0 / SIGMA_SOFT)
    scale = 1.0 - d
    i = np.arange(P)[:, None]
    j = np.arange(P)[None, :]
    e = i - j
    T = np.where(e >= 0, scale * d ** np.clip(e, 0, None), 0.0)
    i64 = np.arange(HIST)[:, None]
    j64 = np.arange(HIST)[None, :]
    with np.errstate(under="ignore"):
        U = scale * d ** (i64 + (HIST - j64))
    ky = _taps64()
    R = 12
    B0 = np.where(np.abs(e) <= R, ky[np.clip(e + R, 0, 2 * R)], 0.0)
    i32 = np.arange(32)[:, None]
    j32 = np.arange(32)[None, :]
    eu = i32 + 32 - j32
    Bup = np.where(np.abs(eu) <= R, ky[np.clip(eu + R, 0, 2 * R)], 0.0)
    ed = i32 - 32 - j32
    Bdn = np.where(np.abs(ed) <= R, ky[np.clip(ed + R, 0, 2 * R)], 0.0)
    f = lambda a: np.ascontiguousarray(a, np.float32)
    return f(T), f(U), f(B0), f(Bup), f(Bdn)


def _build_nc(Hk, wslab, ws):
    """Build the SPMD Bass program for a (Hk, wslab*3) input slab producing
    the central (Hk, ws*3) output."""
    import concourse.bacc as bacc
    import concourse.mybir as mybir
    from concourse.tile import TileContext

    f32 = mybir.dt.float32
    AO = mybir.AluOpType
    nb = Hk // P
    F = wslab * 3
    FC = ws * 3
    OFF = HALO * 3
    HV0 = OFF - 3 * RH          # first x-col (flat) needed for hardv
    FH = FC + 6 * RH            # hardv width
    FHPAD = -(-FH * 4 // 2048) * 512  # pad hardv psum tile to whole banks

    ky = _taps64()
    k0 = float(ky[12])
    cr = [float(ky[12 + t] / ky[12]) for t in range(1, RH + 1)]

    T, U, B0, Bup, Bdn = _matrices()
    wconst_np = np.zeros((128, 384), np.float32)
    wconst_np[:, 0:128] = T.T
    wconst_np[:, 128:256] = B0.T
    wconst_np[64:128, 256:320] = U.T
    wconst_np[96:128, 320:352] = Bup.T
    wconst_np[0:32, 352:384] = Bdn.T

    nc = bacc.Bacc(trn_type="TRN2", debug=False)
    hx = nc.dram_tensor("x", [Hk, F], f32, kind="ExternalInput")
    hcm = nc.dram_tensor("cmat", [1, 9], f32, kind="ExternalInput")
    hy = nc.dram_tensor("y", [Hk, FC], f32, kind="ExternalOutput")
    hconst = nc.inline_tensor(wconst_np, name="wconst")

    def chunks(width):
        out = []
        o = 0
        while o < width:
            out.append((o, min(MMCHUNK, width - o)))
            o += MMCHUNK
        return out

    with TileContext(nc) as tc:
        with tc.tile_pool(name="wpool", bufs=1) as wpool, \
             tc.tile_pool(name="cps_pool", bufs=1, space="PSUM") as cpsp, \
             tc.tile_pool(name="xpool", bufs=4) as xpool, \
             tc.tile_pool(name="hvpool", bufs=2) as hvpool, \
             tc.tile_pool(name="wk", bufs=2) as wk, \
             tc.tile_pool(name="pspool", bufs=1, space="PSUM") as pspool:

            wconst = wpool.tile([128, 384], f32, name="wconst_t")
            nc.sync.dma_start(out=wconst, in_=hconst[:, :])
            wT = wconst[:, 0:128]
            wB = wconst[:, 128:256]
            wU = wconst[64:128, 256:320]
            wBup = wconst[96:128, 320:352]
            wBdn = wconst[0:32, 352:384]

            cmsb = wpool.tile([1, 9], f32, name="cmsb")
            nc.sync.dma_start(out=cmsb, in_=hcm[:, :])
            ones_t = wpool.tile([1, 128], f32, name="ones_t")
            nc.vector.memset(ones_t, 1.0)
            cps = cpsp.tile([128, 16], f32, name="cps")
            nc.tensor.matmul(out=cps[:, 0:9], lhsT=ones_t, rhs=cmsb,
                             start=True, stop=True)
            negc = wpool.tile([128, 16], f32, name="negc")
            nc.scalar.mul(negc[:, 0:9], cps[:, 0:9], -1.0)

            x_tiles = [None] * nb

            def load(b):
                xt = xpool.tile([128, F], f32, name=f"x{b}", tag="x")
                nc.sync.dma_start(out=xt, in_=hx[b * P:(b + 1) * P, :])
                x_tiles[b] = xt

            def process(b):
                xb = x_tiles[b]
                xp = x_tiles[b - 1] if b > 0 else None
                xn = x_tiles[b + 1] if b + 1 < nb else None

                ps_s = pspool.tile([128, FC], f32, name=f"ps_s{b}", tag="ps_s")
                for (o, wdt) in chunks(FC):
                    c0 = OFF + o
                    nc.tensor.matmul(out=ps_s[:, o:o + wdt], lhsT=wT,
                                     rhs=xb[:, c0:c0 + wdt],
                                     start=True, stop=(xp is None))
                    if xp is not None:
                        nc.tensor.matmul(out=ps_s[0:64, o:o + wdt], lhsT=wU,
                                         rhs=xp[64:128, c0:c0 + wdt],
                                         start=False, stop=True,
                                         tile_position=(64, 0))

                ps_h = pspool.tile([128, FHPAD], f32, name=f"ps_h{b}", tag="ps_h")
                for (o, wdt) in chunks(FH):
                    r0 = HV0 + o
                    nc.tensor.matmul(out=ps_h[:, o:o + wdt], lhsT=wB,
                                     rhs=xb[:, r0:r0 + wdt],
                                     start=True,
                                     stop=(xp is None and xn is None))
                    if xp is not None:
                        nc.tensor.matmul(out=ps_h[0:32, o:o + wdt], lhsT=wBup,
                                         rhs=xp[96:128, r0:r0 + wdt],
                                         start=False, stop=(xn is None),
                                         tile_position=(96, 0))
                    if xn is not None:
                        nc.tensor.matmul(out=ps_h[96:128, o:o + wdt], lhsT=wBdn,
                                         rhs=xn[0:32, r0:r0 + wdt],
                                         start=False, stop=True,
                                         tile_position=(0, 96))

                hv = hvpool.tile([128, FH], f32, name=f"hv{b}", tag="hv")
                nc.scalar.copy(out=hv, in_=ps_h[:, 0:FH])

                # horizontal blur: acc = hv0 + sum_t cr[t]*(hv(-t)+hv(+t))
                ctr = hv[:, 3 * RH:3 * RH + FC]
                acc = wk.tile([128, FC], f32, name=f"acc{b}", tag="acc")
                first = True
                for t in range(1, RH + 1):
                    pt = wk.tile([128, FC], f32, name=f"p{t}_{b}", tag=f"p{t}")
                    nc.vector.tensor_add(
                        out=pt,
                        in0=hv[:, 3 * RH - 3 * t:3 * RH - 3 * t + FC],
                        in1=hv[:, 3 * RH + 3 * t:3 * RH + 3 * t + FC])
                    nc.vector.scalar_tensor_tensor(
                        out=acc, in0=pt, scalar=cr[t - 1],
                        in1=(ctr if first else acc),
                        op0=AO.mult, op1=AO.add)
                    first = False

                # diff = k0*acc - soft ; pp = (x*s)*diff ; u = soft + pp
                diff = wk.tile([128, FC], f32, name=f"diff{b}", tag="diff")
                nc.vector.scalar_tensor_tensor(
                    out=diff, in0=(acc if RH > 0 else ctr), scalar=k0,
                    in1=ps_s[:, 0:FC], op0=AO.mult, op1=AO.subtract)
                pp = wk.tile([128, FC], f32, name=f"pp{b}", tag="pp")
                nc.vector.scalar_tensor_tensor(
                    out=pp, in0=xb[:, OFF:OFF + FC], scalar=SINV, in1=diff,
                    op0=AO.mult, op1=AO.mult)
                u = wk.tile([128, FC], f32, name=f"u{b}", tag="u")
                nc.vector.tensor_add(out=u, in0=ps_s[:, 0:FC], in1=pp)

                # channel mix: v_j = x_j - sum_i C[i,j] u_i
                v = wk.tile([128, FC], f32, name=f"v{b}", tag="v")
                ur = u.rearrange("p (w c) -> p c w", c=3)
                xr = xb[:, OFF:OFF + FC].rearrange("p (w c) -> p c w", c=3)
                vr = v.rearrange("p (w c) -> p c w", c=3)
                for j in range(3):
                    for i in range(3):
                        nc.vector.scalar_tensor_tensor(
                            out=vr[:, j, :], in0=ur[:, i, :],
                            scalar=negc[:, 3 * i + j:3 * i + j + 1],
                            in1=(xr[:, j, :] if i == 0 else vr[:, j, :]),
                            op0=AO.mult, op1=AO.add)

                # out = 3*tanh(s*v)
                ot = wk.tile([128, FC], f32, name=f"o{b}", tag="o")
                nc.scalar.activation(out=ot, in_=v,
                                     func=mybir.ActivationFunctionType.Tanh,
                                     scale=SINV)
                nc.vector.tensor_scalar_mul(out=ot, in0=ot, scalar1=3.0)
                nc.sync.dma_start(out=hy[b * P:(b + 1) * P, :], in_=ot)

            load(0)
            if nb > 1:
                load(1)
            for b in range(nb):
                if b + 2 < nb:
                    load(b + 2)
                process(b)

    nc.finalize()
    return nc


def _get_nc(Hk, wslab, ws):
    key = (Hk, wslab, ws)
    if key not in _NC_CACHE:
        _NC_CACHE[key] = _build_nc(Hk, wslab, ws)
    return _NC_CACHE[key]


def kernel(D_macro, coupling_matrix):
    from concourse.bass_utils import run_bass_kernel_spmd

    D = np.asarray(D_macro, dtype=np.float32)
    C = np.ascontiguousarray(np.asarray(coupling_matrix, np.float32).reshape(1, 9))
    Hk, Wk, _ = D.shape
    ws = Wk // NCORES
    wslab = ws + 2 * HALO
    Dp = np.pad(D, ((0, 0), (HALO, HALO), (0, 0)))
    in_maps = []
    for m in range(NCORES):
        sl = np.ascontiguousarray(
            Dp[:, m * ws:m * ws + wslab, :]).reshape(Hk, wslab * 3)
        in_maps.append({"x": sl, "cmat": C})
    nc = _get_nc(Hk, wslab, ws)
    res = run_bass_kernel_spmd(nc, in_maps, core_ids=list(range(NCORES)))
    outs = [r["y"].reshape(Hk, ws, 3) for r in res.results]
    return np.concatenate(outs, axis=1)
